# revision 15
# baseline (speedup 1.0000x reference)
"""Trainium2 Bass kernel for nn_LmLSTMSharedEmb.

Model: tied-embedding LM with 2-layer masked LSTM.
  x = emb[tokens]; x = LSTM0(x); x = LSTM1(x); x = x @ Wp + bp
  logits = einsum('bse,ve->bsv', x, emb); pad rows -> onehot(0)

Distribution: all 8 cores run the (small) LSTM replicated; the
[B,S,V]-sized logits matmul + output write is sharded over vocab
(V=32000 -> 4000 per core).  Full inputs in, full output out.

Device algorithm notes:
  * Everything transposed: hidden/emb dim on partitions, tokens on the
    free axis in order n = 4*t + b  (t-major).
  * LSTM input contribution Zx = X @ Wk + b is precomputed in bulk into
    HBM; the sequential recurrence only streams Wr (bf16, FWL) through
    the PE per step, z^T stored [gates(16) x (128-part hidden blocks)].
  * Masking (pad tokens carry state through): i/f gates get -/+30.0
    pre-activation penalties at masked steps (folded into Zx via an
    augmented contraction row), which makes c carry exactly; h carries
    via a predicated copy with a broadcast mask.  Masked logit rows are
    zeroed by multiplying x by fmask, and the onehot(0) correction is an
    extra rank-1 contraction row in the logits matmul (only the core
    owning vocab column 0 has a nonzero selector row).
  * Biases are folded in as augmented contraction rows against an
    all-ones row (biases in this model are zero, but handled generally).
"""

import numpy as np
import ml_dtypes

import concourse.bass as bass
import concourse.mybir as mybir
import concourse.tile as tile
from concourse import bacc, bass_utils
from concourse.bass import ds, ts
from concourse.masks import make_identity

F16 = np.float16

B, S, E, H, V = 4, 512, 512, 1024, 32000
G = 4 * H                 # gate width 4096
N = B * S                 # 2048 tokens
P = 128
NCORES = 8
VS = V // NCORES          # 4000 vocab per core
BIG = 30.0                # pre-activation mask penalty
UNROLL = 8
F32 = mybir.dt.float32
BF = mybir.dt.float16
I32 = mybir.dt.int32

EK0 = E // P + 1          # wk0 aug chunks (4 + 1)
EK1 = H // P + 1          # wk1/wp aug chunks (8 + 1)
LGK = E // P + 1          # logits contraction chunks (4 + 1)


def _emit_bulk_zx(nc, tc, pools, wk_dram, kc, rhs_fn, zx, tag):
    """zx[:, m, n] (+DRAM, [P, G//P, N] f32) = sum_k wk[k,:,mslice].T @ rhs(k)."""
    stream, staging, ps_big = pools["stream"], pools["staging"], pools["ps_big"]
    for m in range(G // P):
        wkc = stream.tile([P, kc, P], BF, tag=f"wkc_{tag}")
        nc.sync.dma_start(
            wkc[:], wk_dram[:, :, ts(m, P)].rearrange("k p m -> p k m")
        )
        for nb in range(N // 512):
            ps = ps_big.tile([P, 512], F32, tag="ps_big")
            for k in range(kc):
                nc.tensor.matmul(
                    ps[:], lhsT=wkc[:, k], rhs=rhs_fn(k)[:, ts(nb, 512)],
                    start=(k == 0), stop=(k == kc - 1),
                )
            st = staging.tile([P, 512], F32, tag="staging")
            nc.vector.tensor_copy(out=st[:], in_=ps[:])
            nc.sync.dma_start(zx[:, m, ts(nb, 512)], st[:])


def _emit_lstm_layer(nc, tc, pools, wr_sb, zx, yT, bmT):
    """Sequential recurrence; reads zx (DRAM), writes yT [P, H//P, 4*S] bf16.

    Per step one PSUM bank holds z^T for all 32 gate chunks ([P, 32*B]);
    all elementwise work runs as whole-step strided ops.
    """
    state, small4, small16, ps_z, stream = (
        pools["state"], pools["small4"], pools["small16"], pools["ps_z"],
        pools["stream"],
    )
    HB = H // P  # 8 hidden blocks
    MC = G // P  # 32 gate chunks
    hb = [state.tile([P, HB, B], BF, tag="hstate", name=f"hb{i}") for i in range(2)]
    cb = state.tile([P, HB, B], F32, tag="cstate")
    nc.vector.memset(hb[0][:], 0)
    nc.vector.memset(cb[:], 0)

    Sig = mybir.ActivationFunctionType.Sigmoid
    Tanh = mybir.ActivationFunctionType.Tanh

    with tc.For_i(
        0, B * S, B * UNROLL,
        hint_engines=(
            mybir.EngineType.PE, mybir.EngineType.DVE,
            mybir.EngineType.Activation, mybir.EngineType.SP,
        ),
    ) as c0:
        zxbuf = stream.tile([P, MC, B * UNROLL], F32, tag="zxbuf")
        nc.sync.dma_start(zxbuf[:], zx[:, :, ds(c0, B * UNROLL)])
        for u in range(UNROLL):
            cur, nxt = hb[u % 2], hb[(u + 1) % 2]
            col = c0 + B * u
            zps = ps_z.tile([P, MC * B], F32, tag="zps")  # one PSUM bank
            for mc in range(MC):
                for k in range(HB):
                    nc.tensor.matmul(
                        zps[:, ts(mc, B)], lhsT=wr_sb[:, k, ts(mc, P)],
                        rhs=cur[:, k], start=(k == 0), stop=(k == HB - 1),
                    )
            # z = zps + zx slice, then gates = act(z) on strided views
            z_sb = small16.tile([P, MC, B], F32, tag="z_sb")
            nc.vector.tensor_tensor(
                out=z_sb[:], in0=zps[:].rearrange("p (m b) -> p m b", b=B),
                in1=zxbuf[:, :, ds(B * u, B)], op=mybir.AluOpType.add,
            )
            gt = small16.tile([P, MC, B], F32, tag="gates")
            zv = z_sb[:].rearrange("p (j g) b -> p j (g b)", g=4)
            gv = gt[:].rearrange("p (j g) b -> p j (g b)", g=4)
            nc.scalar.activation(gv[:, :, 0:2 * B], zv[:, :, 0:2 * B], Sig)
            nc.scalar.activation(gv[:, :, 2 * B:3 * B], zv[:, :, 2 * B:3 * B], Tanh)
            nc.scalar.activation(gv[:, :, 3 * B:4 * B], zv[:, :, 3 * B:4 * B], Sig)
            gg = gt[:].rearrange("p (j g) b -> p g j b", g=4)
            iv, fv, cv, ov = gg[:, 0], gg[:, 1], gg[:, 2], gg[:, 3]
            # c = f*c + i*g ; h_new = o*tanh(c)
            ig = small4.tile([P, HB, B], F32, tag="ig")
            nc.vector.tensor_mul(out=ig[:], in0=iv, in1=cv)
            nc.vector.tensor_mul(out=cb[:], in0=fv, in1=cb[:])
            nc.vector.tensor_add(out=cb[:], in0=cb[:], in1=ig[:])
            tct = small4.tile([P, HB, B], F32, tag="tct")
            nc.scalar.activation(tct[:], cb[:], Tanh)
            hn = small4.tile([P, HB, B], F32, tag="hn")
            nc.vector.tensor_mul(out=hn[:], in0=ov, in1=tct[:])
            # select: nxt = mask ? h_new : cur   (bf16)
            bm4 = small4.tile([P, B], mybir.dt.uint8, tag="bm4")
            nc.vector.tensor_copy(out=bm4[:], in_=bmT[:, ds(col, B)])
            nc.vector.tensor_copy(out=nxt[:], in_=cur[:])
            nc.vector.copy_predicated(
                out=nxt[:], mask=bm4[:, None, :].to_broadcast([P, HB, B]),
                data=hn[:],
            )
            nc.vector.tensor_copy(out=yT[:, :, ds(col, B)], in_=nxt[:])


def build_program():
    nc = bacc.Bacc("TRN2", target_bir_lowering=False, debug=False)

    tokens = nc.dram_tensor("tokens", [N, 1], I32, kind="ExternalInput")
    embf = nc.dram_tensor("embf", [V, E], F32, kind="ExternalInput")
    wk0 = nc.dram_tensor("wk0", [EK0, P, G], BF, kind="ExternalInput")
    wr0 = nc.dram_tensor("wr0", [H // P, P, G], BF, kind="ExternalInput")
    wk1 = nc.dram_tensor("wk1", [EK1, P, G], BF, kind="ExternalInput")
    wr1 = nc.dram_tensor("wr1", [H // P, P, G], BF, kind="ExternalInput")
    wp = nc.dram_tensor("wp", [EK1, P, E], BF, kind="ExternalInput")
    embT = nc.dram_tensor("embT", [LGK, P, VS], BF, kind="ExternalInput")
    out = nc.dram_tensor("out", [N, VS], F32, kind="ExternalOutput")

    with tile.TileContext(nc) as tc:
        import contextlib
        with contextlib.ExitStack() as ctx:
            dram = ctx.enter_context(tc.tile_pool(name="dram", bufs=1, space="DRAM"))
            big = ctx.enter_context(tc.tile_pool(name="big", bufs=1))
            ypool = ctx.enter_context(tc.tile_pool(name="ypool", bufs=1))
            mid = ctx.enter_context(tc.tile_pool(name="mid", bufs=1))
            masks = ctx.enter_context(tc.tile_pool(name="masks", bufs=1))
            stream = ctx.enter_context(tc.tile_pool(name="stream", bufs=2))
            staging = ctx.enter_context(tc.tile_pool(name="staging", bufs=3))
            state = ctx.enter_context(tc.tile_pool(name="state", bufs=3))
            small4 = ctx.enter_context(tc.tile_pool(name="small4", bufs=8))
            small16 = ctx.enter_context(tc.tile_pool(name="small16", bufs=8))
            ps_big = ctx.enter_context(tc.tile_pool(name="ps_big", bufs=3, space="PSUM"))
            ps_z = ctx.enter_context(tc.tile_pool(name="ps_z", bufs=4, space="PSUM"))
            gather = ctx.enter_context(tc.tile_pool(name="gather", bufs=2))
            pools = dict(stream=stream, staging=staging, ps_big=ps_big, ps_z=ps_z,
                         state=state, small4=small4, small16=small16)

            # ---- token row + masks -------------------------------------
            tok_row = masks.tile([1, N], I32, tag="rowA")
            nc.sync.dma_start(tok_row[:], tokens[:, :].rearrange("n o -> o n"))
            fm_row = masks.tile([1, N], F32, tag="rowB")
            nc.vector.tensor_scalar(
                fm_row[:], tok_row[:], 0, None, mybir.AluOpType.not_equal
            )
            nfm_row = masks.tile([1, N], F32, tag="rowA")
            nc.vector.tensor_scalar(
                nfm_row[:], fm_row[:], -1.0, 1.0,
                mybir.AluOpType.mult, mybir.AluOpType.add,
            )
            nfm_bf = masks.tile([1, N], BF, tag="rowC")
            nc.vector.tensor_copy(out=nfm_bf[:], in_=nfm_row[:])
            # aug rhs tile: row0 = ones (bias), row1 = 1-fmask (penalty)
            aug = masks.tile([P, N], BF)
            nc.vector.memset(aug[:], 0)
            nc.vector.memset(aug[0:1, :], 1.0)
            nc.sync.dma_start(aug[1:2, :], nfm_bf[:])
            # broadcast fmask along partitions: bmT[p, n] = fmask[n]
            # (PE with a row-0 ones selector: out[m,n] = rhs[0,n])
            e0 = masks.tile([P, P], BF)
            nc.vector.memset(e0[:], 0)
            nc.vector.memset(e0[0:1, :], 1.0)
            fm128 = masks.tile([P, N], BF)
            nc.vector.memset(fm128[:], 0)
            nc.vector.tensor_copy(out=fm128[0:1, :], in_=fm_row[:])
            bmT = masks.tile([P, N], mybir.dt.uint8)
            for nb in range(N // 512):
                psb = ps_big.tile([P, 512], F32, tag="ps_big")
                nc.tensor.matmul(psb[:], lhsT=e0[:], rhs=fm128[:, ts(nb, 512)],
                                 start=True, stop=True)
                nc.vector.tensor_copy(out=bmT[:, ts(nb, 512)], in_=psb[:])

            # ---- embedding gather + transpose -> XT bf16 ---------------
            ident = masks.tile([P, P], F32)
            make_identity(nc, ident[:])
            XT = mid.tile([P, E // P, N], BF, tag="mid")
            for cidx in range(N // P):
                idxt = gather.tile([P, 1], I32, tag="idx")
                nc.sync.dma_start(idxt[:], tokens[ds(cidx * P, P), :])
                xt = gather.tile([P, E], F32, tag="xrow")
                nc.gpsimd.indirect_dma_start(
                    out=xt[:], out_offset=None, in_=embf[:, :],
                    in_offset=bass.IndirectOffsetOnAxis(ap=idxt[:, 0:1], axis=0),
                )
                for e in range(E // P):
                    pst = ps_z.tile([P, P], F32, tag="zps")
                    nc.tensor.transpose(pst[:], xt[:, ts(e, P)], ident[:])
                    nc.vector.tensor_copy(
                        out=XT[:, e, ds(cidx * P, P)], in_=pst[:]
                    )

            # ---- layer 0 ----------------------------------------------
            zx = dram.tile([P, G // P, N], F32, tag="zx")
            _emit_bulk_zx(
                nc, tc, pools, wk0, EK0,
                lambda k: XT[:, k] if k < E // P else aug[:], zx, "l0",
            )
            wr_sb = big.tile([P, H // P, G], BF, tag="big")
            nc.sync.dma_start(wr_sb[:], wr0[:, :, :].rearrange("k p m -> p k m"))
            y0T = ypool.tile([P, H // P, N], BF, tag="yT")
            _emit_lstm_layer(nc, tc, pools, wr_sb, zx, y0T, bmT)

            # ---- layer 1 ----------------------------------------------
            zx1 = dram.tile([P, G // P, N], F32, tag="zx")
            _emit_bulk_zx(
                nc, tc, pools, wk1, EK1,
                lambda k: y0T[:, k] if k < H // P else aug[:], zx1, "l1",
            )
            wr_sb1 = big.tile([P, H // P, G], BF, tag="big")
            nc.sync.dma_start(wr_sb1[:], wr1[:, :, :].rearrange("k p m -> p k m"))
            y1T = ypool.tile([P, H // P, N], BF, tag="yT")
            _emit_lstm_layer(nc, tc, pools, wr_sb1, zx1, y1T, bmT)

            # ---- projection: xmT = fmask * (Wp.T @ y1T + bp) -----------
            wp_sb = mid.tile([P, EK1, E], BF, tag="mid")
            nc.sync.dma_start(wp_sb[:], wp[:, :, :].rearrange("k p m -> p k m"))
            xmT = mid.tile([P, E // P, N], BF, tag="mid2")
            nc.vector.memset(xmT[:], 0)
            for m in range(E // P):
                for nb in range(N // 512):
                    ps = ps_big.tile([P, 512], F32, tag="ps_big")
                    for k in range(EK1):
                        rhs = y1T[:, k] if k < H // P else aug[:]
                        nc.tensor.matmul(
                            ps[:], lhsT=wp_sb[:, k, ts(m, P)],
                            rhs=rhs[:, ts(nb, 512)],
                            start=(k == 0), stop=(k == EK1 - 1),
                        )
                    nc.vector.copy_predicated(
                        out=xmT[:, m, ts(nb, 512)], mask=bmT[:, ts(nb, 512)],
                        data=ps[:],
                    )

            # ---- logits: out = xmT_aug.T @ embT_aug --------------------
            embT_sb = big.tile([P, LGK, VS], BF, tag="big")
            nc.sync.dma_start(embT_sb[:], embT[:, :, :].rearrange("k p m -> p k m"))
            NVB = (VS + 511) // 512
            for m in range(N // P):
                for nb in range(NVB):
                    nsz = min(512, VS - nb * 512)
                    ps = ps_big.tile([P, 512], F32, tag="ps_big")
                    for k in range(LGK):
                        lhs = xmT[:, k] if k < E // P else aug[:]
                        nc.tensor.matmul(
                            ps[:, :nsz], lhsT=lhs[:, ts(m, P)],
                            rhs=embT_sb[:, k, ds(nb * 512, nsz)],
                            start=(k == 0), stop=(k == LGK - 1),
                        )
                    st = staging.tile([P, 512], F32, tag="staging")
                    nc.vector.tensor_copy(out=st[:, :nsz], in_=ps[:, :nsz])
                    nc.sync.dma_start(
                        out[ds(m * P, P), ds(nb * 512, nsz)], st[:, :nsz]
                    )

    nc.compile()
    return nc


# ---------------------------------------------------------------------------
# host side
# ---------------------------------------------------------------------------

def _gate_perm():
    """new col (4j+g)*128+u  <-  orig col g*H + j*128 + u."""
    perm = np.empty(G, np.int64)
    for j in range(H // P):
        for g in range(4):
            perm[(4 * j + g) * P:(4 * j + g + 1) * P] = g * H + j * P + np.arange(P)
    return perm


def _prep_inputs(inputs):
    perm = _gate_perm()
    pen = np.zeros(G, np.float32)
    pen[_gate_slice(0)] = -BIG   # i gate
    pen[_gate_slice(1)] = BIG    # f gate

    def aug_w(Wk, b, kc):
        Kd = Wk.shape[0]
        w = np.zeros((kc * P, G), np.float32)
        w[:Kd] = Wk[:, perm]
        w[Kd] = b[perm]
        w[Kd + 1] = pen
        return w.astype(F16).reshape(kc, P, G)

    toks = np.asarray(inputs["inputs"], np.int32)
    tokens_tb = np.ascontiguousarray(toks.T).reshape(N, 1)

    emb = np.asarray(inputs["emb"], np.float32)
    wk0 = aug_w(np.asarray(inputs["Wk0"], np.float32), np.asarray(inputs["b0"], np.float32), EK0)
    wk1 = aug_w(np.asarray(inputs["Wk1"], np.float32), np.asarray(inputs["b1"], np.float32), EK1)

    def chunk(Wr):
        return np.ascontiguousarray(Wr[:, perm]).astype(F16).reshape(H // P, P, G)

    wr0 = chunk(np.asarray(inputs["Wr0"], np.float32))
    wr1 = chunk(np.asarray(inputs["Wr1"], np.float32))

    wp_f = np.asarray(inputs["Wp"], np.float32)
    wpa = np.zeros((EK1 * P, E), np.float32)
    wpa[:H] = wp_f
    wpa[H] = np.asarray(inputs["bp"], np.float32)
    wp = wpa.astype(F16).reshape(EK1, P, E)

    common = dict(tokens=tokens_tb, embf=emb, wk0=wk0, wr0=wr0, wk1=wk1,
                  wr1=wr1, wp=wp)
    in_maps = []
    for c in range(NCORES):
        sl = emb[c * VS:(c + 1) * VS, :].T  # [E, VS]
        # chunk 4 pairs with the `aug` lhsT tile: global row E (aug row0 =
        # ones) must be zero, row E+1 (aug row1 = 1-fmask) is the onehot(0)
        # selector.
        et = np.zeros((LGK * P, VS), np.float32)
        et[:E] = sl
        if c == 0:
            et[E + 1, 0] = 1.0
        in_maps.append(dict(common, embT=et.astype(F16).reshape(LGK, P, VS)))
    return in_maps


def _gate_slice(g):
    idx = np.zeros(G, bool)
    for j in range(H // P):
        idx[(4 * j + g) * P:(4 * j + g + 1) * P] = True
    return idx


_CACHED_NC = None


def _get_nc():
    global _CACHED_NC
    if _CACHED_NC is None:
        _CACHED_NC = build_program()
    return _CACHED_NC


def kernel(**inputs) -> np.ndarray:
    nc = _get_nc()
    in_maps = _prep_inputs(inputs)
    res = bass_utils.run_bass_kernel_spmd(nc, in_maps, core_ids=list(range(NCORES)))
    shards = [np.asarray(res.results[c]["out"], np.float32) for c in range(NCORES)]
    full_tb = np.concatenate(shards, axis=1)          # [N, V], rows n = 4t+b
    return np.ascontiguousarray(
        full_tb.reshape(S, B, V).transpose(1, 0, 2)
    )


# revision 18
# speedup vs baseline: 1.9933x; 1.9933x over previous
"""Trainium2 Bass kernel for nn_LmLSTMSharedEmb.

Model: tied-embedding LM with 2-layer masked LSTM.
  x = emb[tokens]; x = LSTM0(x); x = LSTM1(x); x = x @ Wp + bp
  logits = einsum('bse,ve->bsv', x, emb); pad rows -> onehot(0)

Distribution: all 8 cores run the (small) LSTM replicated; the
[B,S,V]-sized logits matmul + output write is sharded over vocab
(V=32000 -> 4000 per core).  Full inputs in, full output out.

Device algorithm notes:
  * Everything transposed: hidden/emb dim on partitions, tokens on the
    free axis in order n = 4*t + b  (t-major).
  * LSTM input contribution Zx = X @ Wk + b is precomputed in bulk into
    HBM; the sequential recurrence only streams Wr (bf16, FWL) through
    the PE per step, z^T stored [gates(16) x (128-part hidden blocks)].
  * Masking (pad tokens carry state through): i/f gates get -/+30.0
    pre-activation penalties at masked steps (folded into Zx via an
    augmented contraction row), which makes c carry exactly; h carries
    via a predicated copy with a broadcast mask.  Masked logit rows are
    zeroed by multiplying x by fmask, and the onehot(0) correction is an
    extra rank-1 contraction row in the logits matmul (only the core
    owning vocab column 0 has a nonzero selector row).
  * Biases are folded in as augmented contraction rows against an
    all-ones row (biases in this model are zero, but handled generally).
"""

import numpy as np

import concourse.bass as bass
import concourse.mybir as mybir
import concourse.tile as tile
from concourse import bacc, bass_utils
from concourse.bass import ds, ts
from concourse.masks import make_identity

F16 = np.float16

B, S, E, H, V = 4, 512, 512, 1024, 32000
G = 4 * H                 # gate width 4096
N = B * S                 # 2048 tokens
P = 128
NCORES = 8
VS = V // NCORES          # 4000 vocab per core
BIG = 30.0                # pre-activation mask penalty
UNROLL = 8
F32 = mybir.dt.float32
BF = mybir.dt.float16
I32 = mybir.dt.int32

EK0 = E // P + 1          # wk0 aug chunks (4 + 1)
EK1 = H // P + 1          # wk1/wp aug chunks (8 + 1)
LGK = E // P + 1          # logits contraction chunks (4 + 1)


def _emit_bulk_zx(nc, tc, pools, wk_dram, kc, rhs_fn, zx, tag):
    """zx[:, m, n] (+DRAM, [P, G//P, N] f32) = sum_k wk[k,:,mslice].T @ rhs(k)."""
    stream, staging, ps_big = pools["stream"], pools["staging"], pools["ps_big"]
    for m in range(G // P):
        wkc = stream.tile([P, kc, P], BF, tag=f"wkc_{tag}")
        nc.sync.dma_start(
            wkc[:], wk_dram[:, :, ts(m, P)].rearrange("k p m -> p k m")
        )
        for nb in range(N // 512):
            ps = ps_big.tile([P, 512], F32, tag="ps_big")
            for k in range(kc):
                nc.tensor.matmul(
                    ps[:], lhsT=wkc[:, k], rhs=rhs_fn(k)[:, ts(nb, 512)],
                    start=(k == 0), stop=(k == kc - 1),
                )
            st = staging.tile([P, 512], F32, tag="staging")
            nc.vector.tensor_copy(out=st[:], in_=ps[:])
            nc.sync.dma_start(zx[:, m, ts(nb, 512)], st[:])


def _emit_lstm_layer(nc, tc, pools, wr_sb, zx, yT, bmT):
    """Sequential recurrence; reads zx (DRAM), writes yT [P, H//P, 4*S] bf16.

    Per step one PSUM bank holds z^T for all 32 gate chunks ([P, 32*B]);
    all elementwise work runs as whole-step strided ops.
    """
    state, small4, small16, ps_z, stream = (
        pools["state"], pools["small4"], pools["small16"], pools["ps_z"],
        pools["stream"],
    )
    HB = H // P  # 8 hidden blocks
    MC = G // P  # 32 gate chunks
    hb = [state.tile([P, HB, B], BF, tag="hstate", name=f"hb{i}") for i in range(2)]
    cb = state.tile([P, HB, B], F32, tag="cstate")
    nc.vector.memset(hb[0][:], 0)
    nc.vector.memset(cb[:], 0)

    Sig = mybir.ActivationFunctionType.Sigmoid
    Tanh = mybir.ActivationFunctionType.Tanh

    with tc.For_i(
        0, B * S, B * UNROLL,
        hint_engines=(
            mybir.EngineType.PE, mybir.EngineType.DVE,
            mybir.EngineType.Activation, mybir.EngineType.SP,
        ),
    ) as c0:
        zxbuf = stream.tile([P, MC, B * UNROLL], F32, tag="zxbuf")
        nc.sync.dma_start(zxbuf[:], zx[:, :, ds(c0, B * UNROLL)])
        for u in range(UNROLL):
            cur, nxt = hb[u % 2], hb[(u + 1) % 2]
            col = c0 + B * u
            zps = ps_z.tile([P, MC * B], F32, tag="zps")  # one PSUM bank
            for mc in range(MC):
                for k in range(HB):
                    nc.tensor.matmul(
                        zps[:, ts(mc, B)], lhsT=wr_sb[:, k, ts(mc, P)],
                        rhs=cur[:, k], start=(k == 0), stop=(k == HB - 1),
                    )
            # z = zps + zx slice, then gates = act(z) on strided views
            z_sb = small16.tile([P, MC, B], F32, tag="z_sb")
            nc.vector.tensor_tensor(
                out=z_sb[:], in0=zps[:].rearrange("p (m b) -> p m b", b=B),
                in1=zxbuf[:, :, ds(B * u, B)], op=mybir.AluOpType.add,
            )
            gt = small16.tile([P, MC, B], F32, tag="gates")
            zv = z_sb[:].rearrange("p (j g) b -> p j (g b)", g=4)
            gv = gt[:].rearrange("p (j g) b -> p j (g b)", g=4)
            nc.scalar.activation(gv[:, :, 0:2 * B], zv[:, :, 0:2 * B], Sig)
            nc.scalar.activation(gv[:, :, 2 * B:3 * B], zv[:, :, 2 * B:3 * B], Tanh)
            nc.scalar.activation(gv[:, :, 3 * B:4 * B], zv[:, :, 3 * B:4 * B], Sig)
            gg = gt[:].rearrange("p (j g) b -> p g j b", g=4)
            iv, fv, cv, ov = gg[:, 0], gg[:, 1], gg[:, 2], gg[:, 3]
            # c = f*c + i*g ; h_new = o*tanh(c)
            ig = small4.tile([P, HB, B], F32, tag="ig")
            nc.vector.tensor_mul(out=ig[:], in0=iv, in1=cv)
            nc.vector.tensor_mul(out=cb[:], in0=fv, in1=cb[:])
            nc.vector.tensor_add(out=cb[:], in0=cb[:], in1=ig[:])
            tct = small4.tile([P, HB, B], F32, tag="tct")
            nc.scalar.activation(tct[:], cb[:], Tanh)
            hn = small4.tile([P, HB, B], F32, tag="hn")
            nc.vector.tensor_mul(out=hn[:], in0=ov, in1=tct[:])
            # select: nxt = mask ? h_new : cur   (bf16)
            bm4 = small4.tile([P, B], mybir.dt.uint8, tag="bm4")
            nc.vector.tensor_copy(out=bm4[:], in_=bmT[:, ds(col, B)])
            nc.vector.tensor_copy(out=nxt[:], in_=cur[:])
            nc.vector.copy_predicated(
                out=nxt[:], mask=bm4[:, None, :].to_broadcast([P, HB, B]),
                data=hn[:],
            )
            nc.vector.tensor_copy(out=yT[:, :, ds(col, B)], in_=nxt[:])


def build_program():
    nc = bacc.Bacc("TRN2", target_bir_lowering=False, debug=False)

    tokens = nc.dram_tensor("tokens", [N, 1], I32, kind="ExternalInput")
    embf = nc.dram_tensor("embf", [V, E], F32, kind="ExternalInput")
    wk0 = nc.dram_tensor("wk0", [EK0, P, G], BF, kind="ExternalInput")
    wr0 = nc.dram_tensor("wr0", [H // P, P, G], BF, kind="ExternalInput")
    wk1 = nc.dram_tensor("wk1", [EK1, P, G], BF, kind="ExternalInput")
    wr1 = nc.dram_tensor("wr1", [H // P, P, G], BF, kind="ExternalInput")
    wp = nc.dram_tensor("wp", [EK1, P, E], BF, kind="ExternalInput")
    embT = nc.dram_tensor("embT", [LGK, P, VS], BF, kind="ExternalInput")
    out = nc.dram_tensor("out", [N, VS], F32, kind="ExternalOutput")

    with tile.TileContext(nc) as tc:
        import contextlib
        with contextlib.ExitStack() as ctx:
            dram = ctx.enter_context(tc.tile_pool(name="dram", bufs=1, space="DRAM"))
            big = ctx.enter_context(tc.tile_pool(name="big", bufs=1))
            ypool = ctx.enter_context(tc.tile_pool(name="ypool", bufs=1))
            mid = ctx.enter_context(tc.tile_pool(name="mid", bufs=1))
            masks = ctx.enter_context(tc.tile_pool(name="masks", bufs=1))
            stream = ctx.enter_context(tc.tile_pool(name="stream", bufs=2))
            staging = ctx.enter_context(tc.tile_pool(name="staging", bufs=3))
            state = ctx.enter_context(tc.tile_pool(name="state", bufs=3))
            small4 = ctx.enter_context(tc.tile_pool(name="small4", bufs=8))
            small16 = ctx.enter_context(tc.tile_pool(name="small16", bufs=8))
            ps_big = ctx.enter_context(tc.tile_pool(name="ps_big", bufs=3, space="PSUM"))
            ps_z = ctx.enter_context(tc.tile_pool(name="ps_z", bufs=4, space="PSUM"))
            gather = ctx.enter_context(tc.tile_pool(name="gather", bufs=2))
            pools = dict(stream=stream, staging=staging, ps_big=ps_big, ps_z=ps_z,
                         state=state, small4=small4, small16=small16)

            # ---- token row + masks -------------------------------------
            tok_row = masks.tile([1, N], I32, tag="rowA")
            nc.sync.dma_start(tok_row[:], tokens[:, :].rearrange("n o -> o n"))
            fm_row = masks.tile([1, N], F32, tag="rowB")
            nc.vector.tensor_scalar(
                fm_row[:], tok_row[:], 0, None, mybir.AluOpType.not_equal
            )
            nfm_row = masks.tile([1, N], F32, tag="rowA")
            nc.vector.tensor_scalar(
                nfm_row[:], fm_row[:], -1.0, 1.0,
                mybir.AluOpType.mult, mybir.AluOpType.add,
            )
            nfm_bf = masks.tile([1, N], BF, tag="rowC")
            nc.vector.tensor_copy(out=nfm_bf[:], in_=nfm_row[:])
            # aug rhs tile: row0 = ones (bias), row1 = 1-fmask (penalty)
            aug = masks.tile([P, N], BF)
            nc.vector.memset(aug[:], 0)
            nc.vector.memset(aug[0:1, :], 1.0)
            nc.sync.dma_start(aug[1:2, :], nfm_bf[:])
            # broadcast fmask along partitions: bmT[p, n] = fmask[n]
            # (PE with a row-0 ones selector: out[m,n] = rhs[0,n])
            e0 = masks.tile([P, P], BF)
            nc.vector.memset(e0[:], 0)
            nc.vector.memset(e0[0:1, :], 1.0)
            fm128 = masks.tile([P, N], BF)
            nc.vector.memset(fm128[:], 0)
            nc.vector.tensor_copy(out=fm128[0:1, :], in_=fm_row[:])
            bmT = masks.tile([P, N], mybir.dt.uint8)
            for nb in range(N // 512):
                psb = ps_big.tile([P, 512], F32, tag="ps_big")
                nc.tensor.matmul(psb[:], lhsT=e0[:], rhs=fm128[:, ts(nb, 512)],
                                 start=True, stop=True)
                nc.vector.tensor_copy(out=bmT[:, ts(nb, 512)], in_=psb[:])

            # ---- embedding gather + transpose -> XT bf16 ---------------
            ident = masks.tile([P, P], F32)
            make_identity(nc, ident[:])
            XT = mid.tile([P, E // P, N], BF, tag="mid")
            for cidx in range(N // P):
                idxt = gather.tile([P, 1], I32, tag="idx")
                nc.sync.dma_start(idxt[:], tokens[ds(cidx * P, P), :])
                xt = gather.tile([P, E], F32, tag="xrow")
                nc.gpsimd.indirect_dma_start(
                    out=xt[:], out_offset=None, in_=embf[:, :],
                    in_offset=bass.IndirectOffsetOnAxis(ap=idxt[:, 0:1], axis=0),
                )
                for e in range(E // P):
                    pst = ps_z.tile([P, P], F32, tag="zps")
                    nc.tensor.transpose(pst[:], xt[:, ts(e, P)], ident[:])
                    nc.vector.tensor_copy(
                        out=XT[:, e, ds(cidx * P, P)], in_=pst[:]
                    )

            # ---- layer 0 ----------------------------------------------
            zx = dram.tile([P, G // P, N], F32, tag="zx")
            _emit_bulk_zx(
                nc, tc, pools, wk0, EK0,
                lambda k: XT[:, k] if k < E // P else aug[:], zx, "l0",
            )
            wr_sb = big.tile([P, H // P, G], BF, tag="big")
            nc.sync.dma_start(wr_sb[:], wr0[:, :, :].rearrange("k p m -> p k m"))
            y0T = ypool.tile([P, H // P, N], BF, tag="yT")
            _emit_lstm_layer(nc, tc, pools, wr_sb, zx, y0T, bmT)

            # ---- layer 1 ----------------------------------------------
            zx1 = dram.tile([P, G // P, N], F32, tag="zx")
            _emit_bulk_zx(
                nc, tc, pools, wk1, EK1,
                lambda k: y0T[:, k] if k < H // P else aug[:], zx1, "l1",
            )
            wr_sb1 = big.tile([P, H // P, G], BF, tag="big")
            nc.sync.dma_start(wr_sb1[:], wr1[:, :, :].rearrange("k p m -> p k m"))
            y1T = ypool.tile([P, H // P, N], BF, tag="yT")
            _emit_lstm_layer(nc, tc, pools, wr_sb1, zx1, y1T, bmT)

            # ---- projection: xmT = fmask * (Wp.T @ y1T + bp) -----------
            wp_sb = mid.tile([P, EK1, E], BF, tag="mid")
            nc.sync.dma_start(wp_sb[:], wp[:, :, :].rearrange("k p m -> p k m"))
            xmT = mid.tile([P, E // P, N], BF, tag="mid2")
            nc.vector.memset(xmT[:], 0)
            for m in range(E // P):
                for nb in range(N // 512):
                    ps = ps_big.tile([P, 512], F32, tag="ps_big")
                    for k in range(EK1):
                        rhs = y1T[:, k] if k < H // P else aug[:]
                        nc.tensor.matmul(
                            ps[:], lhsT=wp_sb[:, k, ts(m, P)],
                            rhs=rhs[:, ts(nb, 512)],
                            start=(k == 0), stop=(k == EK1 - 1),
                        )
                    nc.vector.copy_predicated(
                        out=xmT[:, m, ts(nb, 512)], mask=bmT[:, ts(nb, 512)],
                        data=ps[:],
                    )

            # ---- logits: out = xmT_aug.T @ embT_aug --------------------
            embT_sb = big.tile([P, LGK, VS], BF, tag="big")
            nc.sync.dma_start(embT_sb[:], embT[:, :, :].rearrange("k p m -> p k m"))
            NVB = (VS + 511) // 512
            for m in range(N // P):
                for nb in range(NVB):
                    nsz = min(512, VS - nb * 512)
                    ps = ps_big.tile([P, 512], F32, tag="ps_big")
                    for k in range(LGK):
                        lhs = xmT[:, k] if k < E // P else aug[:]
                        nc.tensor.matmul(
                            ps[:, :nsz], lhsT=lhs[:, ts(m, P)],
                            rhs=embT_sb[:, k, ds(nb * 512, nsz)],
                            start=(k == 0), stop=(k == LGK - 1),
                        )
                    st = staging.tile([P, 512], F32, tag="staging")
                    nc.vector.tensor_copy(out=st[:, :nsz], in_=ps[:, :nsz])
                    nc.sync.dma_start(
                        out[ds(m * P, P), ds(nb * 512, nsz)], st[:, :nsz]
                    )

    nc.compile()
    return nc


# ---------------------------------------------------------------------------
# host side
# ---------------------------------------------------------------------------

def _gate_perm():
    """new col (4j+g)*128+u  <-  orig col g*H + j*128 + u."""
    perm = np.empty(G, np.int64)
    for j in range(H // P):
        for g in range(4):
            perm[(4 * j + g) * P:(4 * j + g + 1) * P] = g * H + j * P + np.arange(P)
    return perm


def _prep_inputs(inputs):
    perm = _gate_perm()
    pen = np.zeros(G, np.float32)
    pen[_gate_slice(0)] = -BIG   # i gate
    pen[_gate_slice(1)] = BIG    # f gate

    def aug_w(Wk, b, kc):
        Kd = Wk.shape[0]
        w = np.zeros((kc * P, G), np.float32)
        w[:Kd] = Wk[:, perm]
        w[Kd] = b[perm]
        w[Kd + 1] = pen
        return w.astype(F16).reshape(kc, P, G)

    toks = np.asarray(inputs["inputs"], np.int32)
    tokens_tb = np.ascontiguousarray(toks.T).reshape(N, 1)

    emb = np.asarray(inputs["emb"], np.float32)
    wk0 = aug_w(np.asarray(inputs["Wk0"], np.float32), np.asarray(inputs["b0"], np.float32), EK0)
    wk1 = aug_w(np.asarray(inputs["Wk1"], np.float32), np.asarray(inputs["b1"], np.float32), EK1)

    def chunk(Wr):
        return np.ascontiguousarray(Wr[:, perm]).astype(F16).reshape(H // P, P, G)

    wr0 = chunk(np.asarray(inputs["Wr0"], np.float32))
    wr1 = chunk(np.asarray(inputs["Wr1"], np.float32))

    wp_f = np.asarray(inputs["Wp"], np.float32)
    wpa = np.zeros((EK1 * P, E), np.float32)
    wpa[:H] = wp_f
    wpa[H] = np.asarray(inputs["bp"], np.float32)
    wp = wpa.astype(F16).reshape(EK1, P, E)

    common = dict(tokens=tokens_tb, embf=emb, wk0=wk0, wr0=wr0, wk1=wk1,
                  wr1=wr1, wp=wp)
    in_maps = []
    for c in range(NCORES):
        sl = emb[c * VS:(c + 1) * VS, :].T  # [E, VS]
        # chunk 4 pairs with the `aug` lhsT tile: global row E (aug row0 =
        # ones) must be zero, row E+1 (aug row1 = 1-fmask) is the onehot(0)
        # selector.
        et = np.zeros((LGK * P, VS), np.float32)
        et[:E] = sl
        if c == 0:
            et[E + 1, 0] = 1.0
        in_maps.append(dict(common, embT=et.astype(F16).reshape(LGK, P, VS)))
    return in_maps


def _gate_slice(g):
    idx = np.zeros(G, bool)
    for j in range(H // P):
        idx[(4 * j + g) * P:(4 * j + g + 1) * P] = True
    return idx


_CACHED_NC = None


def _get_nc():
    global _CACHED_NC
    if _CACHED_NC is None:
        _CACHED_NC = build_program()
    return _CACHED_NC


def kernel(**inputs) -> np.ndarray:
    nc = _get_nc()
    in_maps = _prep_inputs(inputs)
    res = bass_utils.run_bass_kernel_spmd(nc, in_maps, core_ids=list(range(NCORES)))
    shards = [np.asarray(res.results[c]["out"], np.float32) for c in range(NCORES)]
    full_tb = np.concatenate(shards, axis=1)          # [N, V], rows n = 4t+b
    return np.ascontiguousarray(
        full_tb.reshape(S, B, V).transpose(1, 0, 2)
    )


# revision 19
# speedup vs baseline: 3.4283x; 1.7199x over previous
"""Trainium2 Bass kernel for nn_LmLSTMSharedEmb.

Model: tied-embedding LM with 2-layer masked LSTM.
  x = emb[tokens]; x = LSTM0(x); x = LSTM1(x); x = x @ Wp + bp
  logits = einsum('bse,ve->bsv', x, emb); pad rows -> onehot(0)

Distribution: all 8 cores run the (small) LSTM replicated; the
[B,S,V]-sized logits matmul + output write is sharded over vocab
(V=32000 -> 4000 per core).  Full inputs in, full output out.

Device algorithm notes:
  * Everything transposed: hidden/emb dim on partitions, tokens on the
    free axis in order n = 4*t + b  (t-major).
  * LSTM input contribution Zx = X @ Wk + b is precomputed in bulk into
    HBM; the sequential recurrence only streams Wr (fp16, fast weight
    load) through the PE per step; one PSUM bank holds z^T for all 32
    gate chunks so the elementwise work is a few whole-step strided ops.
  * Masking (pad tokens carry state through): i/f gates get -/+30.0
    pre-activation penalties at masked steps (folded into Zx via an
    augmented contraction row), which makes c carry exactly; h carries
    via a predicated copy with a broadcast mask.  Masked logit rows are
    zeroed by multiplying x by fmask, and the onehot(0) correction is an
    extra rank-1 contraction row in the logits matmul (only the core
    owning vocab column 0 has a nonzero selector row).
  * Biases are folded in as augmented contraction rows against an
    all-ones row (biases in this model are zero, but handled generally).
"""

import numpy as np

import concourse.bass as bass
import concourse.mybir as mybir
import concourse.tile as tile
from concourse import bacc, bass_utils
from concourse.bass import ds, ts
from concourse.masks import make_identity

F16 = np.float16

B, S, E, H, V = 4, 512, 512, 1024, 32000
G = 4 * H                 # gate width 4096
N = B * S                 # 2048 tokens
P = 128
NCORES = 8
VS = V // NCORES          # 4000 vocab per core
BIG = 30.0                # pre-activation mask penalty
UNROLL = 8
F32 = mybir.dt.float32
BF = mybir.dt.float16
I32 = mybir.dt.int32

EK0 = E // P + 1          # wk0 aug chunks (4 + 1)
EK1 = H // P + 1          # wk1/wp aug chunks (8 + 1)
LGK = E // P + 1          # logits contraction chunks (4 + 1)


def _emit_bulk_zx(nc, tc, pools, wk_dram, kc, rhs_fn, zx, tag):
    """zx[:, m, n] (+DRAM, [P, G//P, N] f32) = sum_k wk[k,:,mslice].T @ rhs(k)."""
    stream, staging, ps_big = pools["stream"], pools["staging"], pools["ps_big"]
    for m in range(G // P):
        wkc = stream.tile([P, kc, P], BF, tag=f"wkc_{tag}")
        nc.sync.dma_start(
            wkc[:], wk_dram[:, :, ts(m, P)].rearrange("k p m -> p k m")
        )
        for nb in range(N // 512):
            ps = ps_big.tile([P, 512], F32, tag="ps_big")
            for k in range(kc):
                nc.tensor.matmul(
                    ps[:], lhsT=wkc[:, k], rhs=rhs_fn(k)[:, ts(nb, 512)],
                    start=(k == 0), stop=(k == kc - 1),
                )
            st = staging.tile([P, 512], F32, tag="staging")
            nc.vector.tensor_copy(out=st[:], in_=ps[:])
            nc.sync.dma_start(zx[:, m, ts(nb, 512)], st[:])


def _emit_lstm_layer(nc, tc, pools, wr_sb, zx, yT, bmT):
    """Sequential recurrence; reads zx (DRAM), writes yT [P, H//P, 4*S] bf16.

    Per step one PSUM bank holds z^T for all 32 gate chunks ([P, 32*B]);
    all elementwise work runs as whole-step strided ops.
    """
    state, small4, small16, ps_z, stream = (
        pools["state"], pools["small4"], pools["small16"], pools["ps_z"],
        pools["stream"],
    )
    HB = H // P  # 8 hidden blocks
    MC = G // P  # 32 gate chunks
    hb = [state.tile([P, HB, B], BF, tag="hstate", name=f"hb{i}") for i in range(2)]
    cb = state.tile([P, HB, B], F32, tag="cstate")
    nc.vector.memset(hb[0][:], 0)
    nc.vector.memset(cb[:], 0)

    Sig = mybir.ActivationFunctionType.Sigmoid
    Tanh = mybir.ActivationFunctionType.Tanh

    with tc.For_i(
        0, B * S, B * UNROLL,
        hint_engines=(
            mybir.EngineType.PE, mybir.EngineType.DVE,
            mybir.EngineType.Activation, mybir.EngineType.SP,
        ),
    ) as c0:
        zxbuf = stream.tile([P, MC, B * UNROLL], F32, tag="zxbuf")
        nc.sync.dma_start(zxbuf[:], zx[:, :, ds(c0, B * UNROLL)])
        for u in range(UNROLL):
            cur, nxt = hb[u % 2], hb[(u + 1) % 2]
            col = c0 + B * u
            zps = ps_z.tile([P, MC * B], F32, tag="zps")  # one PSUM bank
            for mc in range(MC):
                for k in range(HB):
                    nc.tensor.matmul(
                        zps[:, ts(mc, B)], lhsT=wr_sb[:, k, ts(mc, P)],
                        rhs=cur[:, k], start=(k == 0), stop=(k == HB - 1),
                    )
            # z = zps + zx slice, then gates = act(z) on strided views
            z_sb = small16.tile([P, MC, B], F32, tag="z_sb")
            nc.vector.tensor_tensor(
                out=z_sb[:], in0=zps[:].rearrange("p (m b) -> p m b", b=B),
                in1=zxbuf[:, :, ds(B * u, B)], op=mybir.AluOpType.add,
            )
            gt = small16.tile([P, MC, B], F32, tag="gates")
            zv = z_sb[:].rearrange("p (j g) b -> p j (g b)", g=4)
            gv = gt[:].rearrange("p (j g) b -> p j (g b)", g=4)
            nc.scalar.activation(gv[:, :, 0:2 * B], zv[:, :, 0:2 * B], Sig)
            nc.scalar.activation(gv[:, :, 2 * B:3 * B], zv[:, :, 2 * B:3 * B], Tanh)
            nc.scalar.activation(gv[:, :, 3 * B:4 * B], zv[:, :, 3 * B:4 * B], Sig)
            gg = gt[:].rearrange("p (j g) b -> p g j b", g=4)
            iv, fv, cv, ov = gg[:, 0], gg[:, 1], gg[:, 2], gg[:, 3]
            # c = f*c + i*g ; h_new = o*tanh(c)
            ig = small4.tile([P, HB, B], F32, tag="ig")
            nc.vector.tensor_mul(out=ig[:], in0=iv, in1=cv)
            nc.vector.tensor_mul(out=cb[:], in0=fv, in1=cb[:])
            nc.vector.tensor_add(out=cb[:], in0=cb[:], in1=ig[:])
            tct = small4.tile([P, HB, B], F32, tag="tct")
            nc.scalar.activation(tct[:], cb[:], Tanh)
            hn = small4.tile([P, HB, B], F32, tag="hn")
            nc.vector.tensor_mul(out=hn[:], in0=ov, in1=tct[:])
            # select: nxt = mask ? h_new : cur   (bf16)
            bm4 = small4.tile([P, B], mybir.dt.uint8, tag="bm4")
            nc.vector.tensor_copy(out=bm4[:], in_=bmT[:, ds(col, B)])
            nc.vector.tensor_copy(out=nxt[:], in_=cur[:])
            nc.vector.copy_predicated(
                out=nxt[:], mask=bm4[:, None, :].to_broadcast([P, HB, B]),
                data=hn[:],
            )
            nc.vector.tensor_copy(out=yT[:, :, ds(col, B)], in_=nxt[:])


def build_program():
    nc = bacc.Bacc("TRN2", target_bir_lowering=False, debug=False)

    tokens = nc.dram_tensor("tokens", [N, 1], I32, kind="ExternalInput")
    embf = nc.dram_tensor("embf", [V, E], F32, kind="ExternalInput")
    wk0 = nc.dram_tensor("wk0", [EK0, P, G], BF, kind="ExternalInput")
    wr0 = nc.dram_tensor("wr0", [H // P, P, G], BF, kind="ExternalInput")
    wk1 = nc.dram_tensor("wk1", [EK1, P, G], BF, kind="ExternalInput")
    wr1 = nc.dram_tensor("wr1", [H // P, P, G], BF, kind="ExternalInput")
    wp = nc.dram_tensor("wp", [EK1, P, E], BF, kind="ExternalInput")
    embT = nc.dram_tensor("embT", [LGK, P, VS], BF, kind="ExternalInput")
    out = nc.dram_tensor("out", [N, VS], F32, kind="ExternalOutput")

    with tile.TileContext(nc) as tc:
        import contextlib
        with contextlib.ExitStack() as ctx:
            dram = ctx.enter_context(tc.tile_pool(name="dram", bufs=1, space="DRAM"))
            big = ctx.enter_context(tc.tile_pool(name="big", bufs=1))
            ypool = ctx.enter_context(tc.tile_pool(name="ypool", bufs=1))
            mid = ctx.enter_context(tc.tile_pool(name="mid", bufs=1))
            masks = ctx.enter_context(tc.tile_pool(name="masks", bufs=1))
            stream = ctx.enter_context(tc.tile_pool(name="stream", bufs=2))
            staging = ctx.enter_context(tc.tile_pool(name="staging", bufs=3))
            state = ctx.enter_context(tc.tile_pool(name="state", bufs=3))
            small4 = ctx.enter_context(tc.tile_pool(name="small4", bufs=8))
            small16 = ctx.enter_context(tc.tile_pool(name="small16", bufs=8))
            ps_big = ctx.enter_context(tc.tile_pool(name="ps_big", bufs=3, space="PSUM"))
            ps_z = ctx.enter_context(tc.tile_pool(name="ps_z", bufs=4, space="PSUM"))
            gather = ctx.enter_context(tc.tile_pool(name="gather", bufs=2))
            pools = dict(stream=stream, staging=staging, ps_big=ps_big, ps_z=ps_z,
                         state=state, small4=small4, small16=small16)

            # ---- token row + masks -------------------------------------
            tok_row = masks.tile([1, N], I32, tag="rowA")
            nc.sync.dma_start(tok_row[:], tokens[:, :].rearrange("n o -> o n"))
            fm_row = masks.tile([1, N], F32, tag="rowB")
            nc.vector.tensor_scalar(
                fm_row[:], tok_row[:], 0, None, mybir.AluOpType.not_equal
            )
            nfm_row = masks.tile([1, N], F32, tag="rowA")
            nc.vector.tensor_scalar(
                nfm_row[:], fm_row[:], -1.0, 1.0,
                mybir.AluOpType.mult, mybir.AluOpType.add,
            )
            nfm_bf = masks.tile([1, N], BF, tag="rowC")
            nc.vector.tensor_copy(out=nfm_bf[:], in_=nfm_row[:])
            # aug rhs tile: row0 = ones (bias), row1 = 1-fmask (penalty)
            aug = masks.tile([P, N], BF)
            nc.vector.memset(aug[:], 0)
            nc.vector.memset(aug[0:1, :], 1.0)
            nc.sync.dma_start(aug[1:2, :], nfm_bf[:])
            # broadcast fmask along partitions: bmT[p, n] = fmask[n]
            # (PE with a row-0 ones selector: out[m,n] = rhs[0,n])
            e0 = masks.tile([P, P], BF)
            nc.vector.memset(e0[:], 0)
            nc.vector.memset(e0[0:1, :], 1.0)
            fm128 = masks.tile([P, N], BF)
            nc.vector.memset(fm128[:], 0)
            nc.vector.tensor_copy(out=fm128[0:1, :], in_=fm_row[:])
            bmT = masks.tile([P, N], mybir.dt.uint8)
            for nb in range(N // 512):
                psb = ps_big.tile([P, 512], F32, tag="ps_big")
                nc.tensor.matmul(psb[:], lhsT=e0[:], rhs=fm128[:, ts(nb, 512)],
                                 start=True, stop=True)
                nc.vector.tensor_copy(out=bmT[:, ts(nb, 512)], in_=psb[:])

            # ---- embedding gather + transpose -> XT bf16 ---------------
            ident = masks.tile([P, P], F32)
            make_identity(nc, ident[:])
            XT = mid.tile([P, E // P, N], BF, tag="mid")
            for cidx in range(N // P):
                idxt = gather.tile([P, 1], I32, tag="idx")
                nc.sync.dma_start(idxt[:], tokens[ds(cidx * P, P), :])
                xt = gather.tile([P, E], F32, tag="xrow")
                nc.gpsimd.indirect_dma_start(
                    out=xt[:], out_offset=None, in_=embf[:, :],
                    in_offset=bass.IndirectOffsetOnAxis(ap=idxt[:, 0:1], axis=0),
                )
                for e in range(E // P):
                    pst = ps_z.tile([P, P], F32, tag="zps")
                    nc.tensor.transpose(pst[:], xt[:, ts(e, P)], ident[:])
                    nc.vector.tensor_copy(
                        out=XT[:, e, ds(cidx * P, P)], in_=pst[:]
                    )

            # ---- layer 0 ----------------------------------------------
            zx = dram.tile([P, G // P, N], F32, tag="zx")
            _emit_bulk_zx(
                nc, tc, pools, wk0, EK0,
                lambda k: XT[:, k] if k < E // P else aug[:], zx, "l0",
            )
            wr_sb = big.tile([P, H // P, G], BF, tag="big")
            nc.sync.dma_start(wr_sb[:], wr0[:, :, :].rearrange("k p m -> p k m"))
            y0T = ypool.tile([P, H // P, N], BF, tag="yT")
            _emit_lstm_layer(nc, tc, pools, wr_sb, zx, y0T, bmT)

            # ---- layer 1 ----------------------------------------------
            zx1 = dram.tile([P, G // P, N], F32, tag="zx")
            _emit_bulk_zx(
                nc, tc, pools, wk1, EK1,
                lambda k: y0T[:, k] if k < H // P else aug[:], zx1, "l1",
            )
            wr_sb1 = big.tile([P, H // P, G], BF, tag="big")
            nc.sync.dma_start(wr_sb1[:], wr1[:, :, :].rearrange("k p m -> p k m"))
            y1T = ypool.tile([P, H // P, N], BF, tag="yT")
            _emit_lstm_layer(nc, tc, pools, wr_sb1, zx1, y1T, bmT)

            # ---- projection: xmT = fmask * (Wp.T @ y1T + bp) -----------
            wp_sb = mid.tile([P, EK1, E], BF, tag="mid")
            nc.sync.dma_start(wp_sb[:], wp[:, :, :].rearrange("k p m -> p k m"))
            xmT = mid.tile([P, E // P, N], BF, tag="mid2")
            nc.vector.memset(xmT[:], 0)
            for m in range(E // P):
                for nb in range(N // 512):
                    ps = ps_big.tile([P, 512], F32, tag="ps_big")
                    for k in range(EK1):
                        rhs = y1T[:, k] if k < H // P else aug[:]
                        nc.tensor.matmul(
                            ps[:], lhsT=wp_sb[:, k, ts(m, P)],
                            rhs=rhs[:, ts(nb, 512)],
                            start=(k == 0), stop=(k == EK1 - 1),
                        )
                    nc.vector.copy_predicated(
                        out=xmT[:, m, ts(nb, 512)], mask=bmT[:, ts(nb, 512)],
                        data=ps[:],
                    )

            # ---- logits: out = xmT_aug.T @ embT_aug --------------------
            embT_sb = big.tile([P, LGK, VS], BF, tag="big")
            nc.sync.dma_start(embT_sb[:], embT[:, :, :].rearrange("k p m -> p k m"))
            NVB = (VS + 511) // 512
            for m in range(N // P):
                for nb in range(NVB):
                    nsz = min(512, VS - nb * 512)
                    ps = ps_big.tile([P, 512], F32, tag="ps_big")
                    for k in range(LGK):
                        lhs = xmT[:, k] if k < E // P else aug[:]
                        nc.tensor.matmul(
                            ps[:, :nsz], lhsT=lhs[:, ts(m, P)],
                            rhs=embT_sb[:, k, ds(nb * 512, nsz)],
                            start=(k == 0), stop=(k == LGK - 1),
                        )
                    st = staging.tile([P, 512], F32, tag="staging")
                    nc.vector.tensor_copy(out=st[:, :nsz], in_=ps[:, :nsz])
                    nc.sync.dma_start(
                        out[ds(m * P, P), ds(nb * 512, nsz)], st[:, :nsz]
                    )

    nc.compile()
    return nc


# ---------------------------------------------------------------------------
# host side
# ---------------------------------------------------------------------------

def _gate_perm():
    """new col (4j+g)*128+u  <-  orig col g*H + j*128 + u."""
    perm = np.empty(G, np.int64)
    for j in range(H // P):
        for g in range(4):
            perm[(4 * j + g) * P:(4 * j + g + 1) * P] = g * H + j * P + np.arange(P)
    return perm


def _prep_inputs(inputs):
    perm = _gate_perm()
    pen = np.zeros(G, np.float32)
    pen[_gate_slice(0)] = -BIG   # i gate
    pen[_gate_slice(1)] = BIG    # f gate

    def aug_w(Wk, b, kc):
        Kd = Wk.shape[0]
        w = np.zeros((kc * P, G), np.float32)
        w[:Kd] = Wk[:, perm]
        w[Kd] = b[perm]
        w[Kd + 1] = pen
        return w.astype(F16).reshape(kc, P, G)

    toks = np.asarray(inputs["inputs"], np.int32)
    tokens_tb = np.ascontiguousarray(toks.T).reshape(N, 1)

    emb = np.asarray(inputs["emb"], np.float32)
    wk0 = aug_w(np.asarray(inputs["Wk0"], np.float32), np.asarray(inputs["b0"], np.float32), EK0)
    wk1 = aug_w(np.asarray(inputs["Wk1"], np.float32), np.asarray(inputs["b1"], np.float32), EK1)

    def chunk(Wr):
        return np.ascontiguousarray(Wr[:, perm]).astype(F16).reshape(H // P, P, G)

    wr0 = chunk(np.asarray(inputs["Wr0"], np.float32))
    wr1 = chunk(np.asarray(inputs["Wr1"], np.float32))

    wp_f = np.asarray(inputs["Wp"], np.float32)
    wpa = np.zeros((EK1 * P, E), np.float32)
    wpa[:H] = wp_f
    wpa[H] = np.asarray(inputs["bp"], np.float32)
    wp = wpa.astype(F16).reshape(EK1, P, E)

    common = dict(tokens=tokens_tb, embf=emb, wk0=wk0, wr0=wr0, wk1=wk1,
                  wr1=wr1, wp=wp)
    in_maps = []
    for c in range(NCORES):
        sl = emb[c * VS:(c + 1) * VS, :].T  # [E, VS]
        # chunk 4 pairs with the `aug` lhsT tile: global row E (aug row0 =
        # ones) must be zero, row E+1 (aug row1 = 1-fmask) is the onehot(0)
        # selector.
        et = np.zeros((LGK * P, VS), np.float32)
        et[:E] = sl
        if c == 0:
            et[E + 1, 0] = 1.0
        in_maps.append(dict(common, embT=et.astype(F16).reshape(LGK, P, VS)))
    return in_maps


def _gate_slice(g):
    idx = np.zeros(G, bool)
    for j in range(H // P):
        idx[(4 * j + g) * P:(4 * j + g + 1) * P] = True
    return idx


_CACHED_NC = None


def _get_nc():
    global _CACHED_NC
    if _CACHED_NC is None:
        _CACHED_NC = build_program()
    return _CACHED_NC


def kernel(**inputs) -> np.ndarray:
    nc = _get_nc()
    in_maps = _prep_inputs(inputs)
    res = bass_utils.run_bass_kernel_spmd(nc, in_maps, core_ids=list(range(NCORES)))
    shards = [np.asarray(res.results[c]["out"], np.float32) for c in range(NCORES)]
    full_tb = np.concatenate(shards, axis=1)          # [N, V], rows n = 4t+b
    return np.ascontiguousarray(
        full_tb.reshape(S, B, V).transpose(1, 0, 2)
    )
